# revision 1
# baseline (speedup 1.0000x reference)
# Trainium2 Bass kernel for nn_EncoderRNN (bidirectional LSTM + attention + classifier).
#
# Sharding: data-parallel over batch (B=128 -> 16 per core, 8 cores), both LSTM
# directions computed per core, weights replicated. The sequential time scan
# stays local per shard.
#
# Self-contained: hardcodes shapes; takes full inputs, returns full output.
import numpy as np
import ml_dtypes

B, L, E, H, C = 128, 512, 512, 512, 16
NCORES = 8
BS = B // NCORES          # batch per core
W = 4                     # timesteps per x-precompute window
NW = L // W               # windows
UNROLL = 16               # For_i unroll factor
KC_H = H // 128           # h-part contraction chunks (4)
KC_E = E // 128           # x-part contraction chunks (4)
NMT = 4 * H // 128        # gate M-tiles (16); mt = gg*4 + hc
TOKCH = 16                # attention token chunks (512 tokens each)
TOKL = L // TOKCH         # l-range per token chunk (32)

_cache = {}
DEBUG = False


def _build_nc(rec_reps=1, attn_reps=1):
    import concourse.bacc as bacc
    import concourse.mybir as mybir
    import concourse.tile as tile
    from concourse.bass import ds
    import contextlib

    f32 = mybir.dt.float32
    bf16 = mybir.dt.bfloat16
    AF = mybir.ActivationFunctionType
    ALU = mybir.AluOpType
    AX = mybir.AxisListType

    nc = bacc.Bacc("TRN2", target_bir_lowering=False, debug=False,
                   num_devices=NCORES)

    # ---- I/O ----
    # x pre-transposed on host into window-major layout [NW, E, W, BS]
    xT = nc.dram_tensor("xT", [NW, E, W, BS], bf16, kind="ExternalInput").ap()
    xTr = nc.dram_tensor("xTr", [NW, E, W, BS], bf16, kind="ExternalInput").ap()
    wf = nc.dram_tensor("wf", [E + H, 4 * H], bf16, kind="ExternalInput").ap()
    wb = nc.dram_tensor("wb", [E + H, 4 * H], bf16, kind="ExternalInput").ap()
    bias_blk = nc.dram_tensor("bias_blk", [2, 16, 128], bf16,
                              kind="ExternalInput").ap()
    ind = nc.dram_tensor("ind", [16, 1024], bf16, kind="ExternalInput").ap()
    aw = nc.dram_tensor("aw", [2 * H, 2 * H], bf16, kind="ExternalInput").ap()
    ab_t = nc.dram_tensor("ab_t", [128, 2 * H // 128], f32, kind="ExternalInput").ap()
    av_t = nc.dram_tensor("av_t", [128, 2 * H // 128], bf16, kind="ExternalInput").ap()
    cw = nc.dram_tensor("cw", [2 * H, C], f32, kind="ExternalInput").ap()
    cb_rep = nc.dram_tensor("cb_rep", [BS, C], f32, kind="ExternalInput").ap()
    maskadd = nc.dram_tensor("maskadd", [BS, L], f32, kind="ExternalInput").ap()
    out = nc.dram_tensor("out", [BS, C], f32, kind="ExternalOutput").ap()

    wdr = [wf, wb]
    xv = [xT, xTr]

    with tile.TileContext(nc) as tc:
        with contextlib.ExitStack() as ctx:
            dramp = ctx.enter_context(tc.tile_pool(name="dram", bufs=1, space="DRAM"))
            # hid[ch][p, l, b]; ch = dir*4 + hc (h2 = ch*128 + p)
            if DEBUG:
                hid = nc.dram_tensor("hid_dbg", [8, 128, L, BS], bf16,
                                     kind="ExternalOutput").ap()
                alpha_d = nc.dram_tensor("alpha_dbg", [L, BS], f32,
                                         kind="ExternalOutput").ap()
                attw_d = nc.dram_tensor("attw_dbg", [L, BS], bf16,
                                        kind="ExternalOutput").ap()
            else:
                hid = dramp.tile([8, 128, L, BS], bf16)
                alpha_d = dramp.tile([L, BS], f32)
                attw_d = dramp.tile([L, BS], bf16)

            # ================= Phase B: bidirectional LSTM =================
            with contextlib.ExitStack() as rctx:
                wpool = rctx.enter_context(tc.tile_pool(name="wp", bufs=1))
                xpool = rctx.enter_context(tc.tile_pool(name="xp", bufs=2))
                spool = rctx.enter_context(tc.tile_pool(name="sp", bufs=3))
                ppool = rctx.enter_context(
                    tc.tile_pool(name="pp", bufs=2, space="PSUM"))

                # weights: [128, 8 kc, 2048] per dir (kc 0-3: x, 4-7: h)
                w_sb = []
                for d in range(2):
                    t = wpool.tile([128, 8, 4 * H], bf16, tag=f"w{d}")
                    for kc in range(8):
                        nc.sync.dma_start(
                            out=t[:, kc, :],
                            in_=wdr[d][kc * 128:(kc + 1) * 128, :])
                    w_sb.append(t)
                bb_sb = []
                for d in range(2):
                    t = wpool.tile([16, 128], bf16, tag=f"bb{d}")
                    nc.sync.dma_start(out=t, in_=bias_blk[d])
                    bb_sb.append(t)
                ind_sb = wpool.tile([16, 1024], bf16, tag="ind")
                nc.sync.dma_start(out=ind_sb, in_=ind)

                # recurrent state
                h_bf = []
                c_st = []
                for d in range(2):
                    hbt = wpool.tile([128, KC_H, BS], bf16, tag=f"h{d}")
                    nc.vector.memset(hbt, 0.0)
                    h_bf.append(hbt)
                    cst = wpool.tile([128, KC_H, BS], f32, tag=f"c{d}")
                    nc.vector.memset(cst, 0.0)
                    c_st.append(cst)

                stg_state = {}

                def window(wi, k):
                    psums = []
                    if k % 2 == 0:
                        stg_state["stg"] = [
                            spool.tile([128, KC_H, 2 * W, BS], bf16,
                                       name=f"stg{d}", tag=f"stg{d}")
                            for d in range(2)]
                    stgs = stg_state["stg"]
                    for d in range(2):
                        x_sb = xpool.tile([128, KC_E, W, BS], bf16, tag=f"x{d}")
                        for ec in range(KC_E):
                            nc.sync.dma_start(
                                out=x_sb[:, ec, :, :],
                                in_=xv[d][ds(wi, 1),
                                          ec * 128:(ec + 1) * 128,
                                          :, :].squeeze(0))
                        # psum [128, hc, gg, t, b]
                        ps = ppool.tile([128, KC_H, 4, W, BS], f32, tag=f"ps{d}")
                        psums.append(ps)
                        # bank openers: write whole bank (bias values) with
                        # start=True so everything after purely accumulates
                        psflat = ps.rearrange("p hc gg t b -> p (hc gg t b)")
                        for bank in range(2):
                            nc.tensor.matmul(
                                psflat[:, bank * 512:(bank + 1) * 512],
                                bb_sb[d][:, :],
                                ind_sb[:, bank * 512:(bank + 1) * 512],
                                start=True, stop=False, skip_group_check=True)
                        xflat = x_sb.rearrange("p e t b -> p e (t b)")
                        for ec in range(KC_E):
                            for mt in range(NMT):
                                gg, hc = mt // 4, mt % 4
                                nc.tensor.matmul(
                                    ps[:, hc, gg, :, :],
                                    w_sb[d][:, ec, mt * 128:(mt + 1) * 128],
                                    xflat[:, ec, :],
                                    start=False, stop=False,
                                    skip_group_check=True)

                    for ti in range(W):
                        for d in range(2):
                            ps = psums[d]
                            for kc in range(KC_H):
                                for mt in range(NMT):
                                    gg, hc = mt // 4, mt % 4
                                    nc.tensor.matmul(
                                        ps[:, hc, gg, ti, :],
                                        w_sb[d][:, 4 + kc,
                                                mt * 128:(mt + 1) * 128],
                                        h_bf[d][:, kc, :],
                                        start=False, stop=False,
                                        skip_group_check=True)
                            fio = spool.tile([128, KC_H, 3, BS], f32,
                                             tag=f"fio{d}")
                            nc.scalar.activation(fio, ps[:, :, 0:3, ti, :],
                                                 AF.Sigmoid)
                            g_s = spool.tile([128, KC_H, BS], f32, tag=f"g{d}")
                            nc.scalar.activation(g_s, ps[:, :, 3, ti, :],
                                                 AF.Tanh)
                            ig = spool.tile([128, KC_H, BS], f32, tag=f"ig{d}")
                            nc.vector.tensor_mul(ig, fio[:, :, 1, :], g_s)
                            fc = spool.tile([128, KC_H, BS], f32, tag=f"fc{d}")
                            nc.vector.tensor_mul(fc, fio[:, :, 0, :], c_st[d])
                            nc.vector.tensor_add(c_st[d], ig, fc)
                            tc_s = spool.tile([128, KC_H, BS], f32,
                                              tag=f"tc{d}")
                            nc.scalar.activation(tc_s, c_st[d], AF.Tanh)
                            nc.vector.tensor_mul(h_bf[d], fio[:, :, 2, :],
                                                 tc_s)
                            # stage h for the pair-batched hid write; bwd
                            # occupies reversed slots so dst times ascend
                            sl = (k % 2) * W + ti
                            slot = sl if d == 0 else 2 * W - 1 - sl
                            nc.vector.tensor_copy(stgs[d][:, :, slot, :],
                                                  h_bf[d])
                    if k % 2 == 1:
                        wbase = wi - 1  # symbolic start of the pair
                        for d in range(2):
                            td0 = (wbase * W if d == 0
                                   else L - 2 * W - wbase * W)
                            for hc in range(KC_H):
                                nc.sync.dma_start(
                                    out=hid[d * 4 + hc, :, ds(td0, 2 * W), :],
                                    in_=stgs[d][:, hc, :, :])

                def unroll_body(iv0, unroll):
                    assert unroll % 2 == 0, unroll
                    for k in range(unroll):
                        window(iv0 + k, k)

                if rec_reps == 1:
                    tc.For_i_unrolled_general(
                        0, NW, 1, unrollable_body=unroll_body,
                        max_unroll=UNROLL)
                else:
                    with tc.For_i(0, rec_reps) as _r:
                        tc.For_i_unrolled_general(
                            0, NW, 1, unrollable_body=unroll_body,
                            max_unroll=UNROLL)

            # ================= Phase C: attention + classifier =============
            actx_loop = tc.For_i(0, attn_reps) if attn_reps > 1 else None
            if actx_loop is not None:
                actx_loop.__enter__()
            with contextlib.ExitStack() as actx:
                cpool = actx.enter_context(tc.tile_pool(name="cp", bufs=1))
                hpool = actx.enter_context(tc.tile_pool(name="hp", bufs=2))
                apool = actx.enter_context(tc.tile_pool(name="ap", bufs=3))
                mpool = actx.enter_context(tc.tile_pool(name="mp", bufs=1))
                pap = actx.enter_context(
                    tc.tile_pool(name="pap", bufs=2, space="PSUM"))
                pal = actx.enter_context(
                    tc.tile_pool(name="pal", bufs=2, space="PSUM"))

                aw_sb = cpool.tile([128, 8, 2 * H], bf16)
                for kc in range(8):
                    nc.sync.dma_start(out=aw_sb[:, kc, :],
                                      in_=aw[kc * 128:(kc + 1) * 128, :])
                ab_sb = cpool.tile([128, 8], f32)
                nc.sync.dma_start(out=ab_sb, in_=ab_t)
                av_sb = cpool.tile([128, 8], bf16)
                nc.sync.dma_start(out=av_sb, in_=av_t)

                for tck in range(TOKCH):
                    l0 = tck * TOKL
                    hid_sb = hpool.tile([128, 8, TOKL, BS], bf16, tag="hsb")
                    for ch in range(8):
                        nc.sync.dma_start(out=hid_sb[:, ch, :, :],
                                          in_=hid[ch, :, l0:l0 + TOKL, :])
                    hflat = hid_sb.rearrange("p c l b -> p c (l b)")
                    ps_al = pal.tile([1, TOKL * BS], f32, tag="psal")
                    for m in range(8):
                        ps_a = pap.tile([128, TOKL * BS], f32, tag="psa")
                        for kc in range(8):
                            nc.tensor.matmul(
                                ps_a, aw_sb[:, kc, m * 128:(m + 1) * 128],
                                hflat[:, kc, :],
                                start=(kc == 0), stop=(kc == 7))
                        at_sb = apool.tile([128, TOKL * BS], bf16, tag="atsb")
                        nc.scalar.activation(at_sb, ps_a, AF.Tanh,
                                             bias=ab_sb[:, m:m + 1])
                        nc.tensor.matmul(ps_al, av_sb[:, m:m + 1], at_sb,
                                         start=(m == 0), stop=(m == 7))
                    al_sb = apool.tile([1, TOKL * BS], f32, tag="alsb")
                    nc.scalar.copy(al_sb, ps_al)
                    nc.sync.dma_start(
                        out=alpha_d[l0:l0 + TOKL, :],
                        in_=al_sb.rearrange("p (l b) -> p l b", l=TOKL))

                # softmax over l per b
                alv = mpool.tile([BS, L], f32)
                nc.sync.dma_start(out=alv, in_=alpha_d.rearrange("l b -> b l"))
                madd = mpool.tile([BS, L], f32)
                nc.sync.dma_start(out=madd, in_=maskadd)
                alm = mpool.tile([BS, L], f32)
                nc.vector.tensor_add(alm, alv, madd)
                negmax = mpool.tile([BS, 1], f32)
                nc.vector.tensor_reduce(negmax, alm, AX.X, ALU.max,
                                        negate=True)
                esb = mpool.tile([BS, L], f32)
                ssum = mpool.tile([BS, 1], f32)
                nc.scalar.activation(esb, alm, AF.Exp, bias=negmax,
                                     accum_out=ssum)
                rsum = mpool.tile([BS, 1], f32)
                nc.vector.reciprocal(rsum, ssum)
                attw = mpool.tile([BS, L], bf16)
                nc.vector.tensor_scalar_mul(attw, esb, rsum)
                nc.sync.dma_start(out=attw_d.rearrange("l b -> b l"), in_=attw)

                # sent = einsum over l
                import concourse.bass as bass
                attw_flat = attw_d.rearrange("l b -> (l b)")
                attw_bcast = bass.AP(tensor=attw_flat.tensor,
                                     offset=attw_flat.offset,
                                     ap=[[0, 128]] + list(attw_flat.ap))
                attw_rep = mpool.tile([128, L * BS], bf16)
                nc.sync.dma_start(out=attw_rep, in_=attw_bcast)
                arv = attw_rep.rearrange("p (l b) -> p l b", l=L)
                sent = mpool.tile([128, 8, BS], f32)
                for ch in range(8):
                    hfull = hpool.tile([128, L, BS], bf16, tag="hfull")
                    nc.sync.dma_start(out=hfull, in_=hid[ch, :, :, :])
                    mul_t = hpool.tile([128, L, BS], bf16, tag="mult")
                    nc.vector.tensor_mul(mul_t, hfull, arv)
                    nc.vector.tensor_reduce(
                        sent[:, ch, :], mul_t.rearrange("p l b -> p b l"),
                        AX.X, ALU.add)

                # classifier
                cw_sb = cpool.tile([128, 8, C], f32)
                for kc in range(8):
                    nc.sync.dma_start(out=cw_sb[:, kc, :],
                                      in_=cw[kc * 128:(kc + 1) * 128, :])
                cb_sb = cpool.tile([BS, C], f32)
                nc.sync.dma_start(out=cb_sb, in_=cb_rep)
                sent_c = mpool.tile([128, 8, BS], f32)
                nc.vector.tensor_copy(sent_c, sent)
                ps_c = pal.tile([BS, C], f32, tag="psc")
                for ch in range(8):
                    nc.tensor.matmul(ps_c, sent_c[:, ch, :], cw_sb[:, ch, :],
                                     start=(ch == 0), stop=(ch == 7))
                logits = mpool.tile([BS, C], f32)
                nc.vector.tensor_add(logits, ps_c, cb_sb)
                ngm = mpool.tile([BS, 1], f32)
                nc.vector.tensor_reduce(ngm, logits, AX.X, ALU.max,
                                        negate=True)
                e2 = mpool.tile([BS, C], f32)
                s2 = mpool.tile([BS, 1], f32)
                nc.scalar.activation(e2, logits, AF.Exp, bias=ngm,
                                     accum_out=s2)
                lns = mpool.tile([BS, 1], f32)
                nc.scalar.activation(lns, s2, AF.Ln)
                tmp1 = mpool.tile([BS, C], f32)
                nc.vector.tensor_scalar_add(tmp1, logits, ngm)
                res = mpool.tile([BS, C], f32)
                nc.vector.tensor_scalar_sub(res, tmp1, lns)
                nc.sync.dma_start(out=out, in_=res)
            if actx_loop is not None:
                actx_loop.__exit__(None, None, None)

    nc.compile()
    return nc


def _prep_host(x, mask, fWf, fbf, fWi, fbi, fWo, fbo, fWg, fbg,
               bWf, bbf, bWi, bbi, bWo, bbo, bWg, bbg,
               aW, ab, av, cW, cb):
    bf = ml_dtypes.bfloat16

    def aug(Ws):
        # [E+H, 4H]: rows 0..E-1 x-part, E..E+H-1 h-part
        m = np.zeros((E + H, 4 * H), np.float32)
        for g, Wg_ in enumerate(Ws):
            m[:, g * H:(g + 1) * H] = Wg_
        return m.astype(bf)

    def bias_block(bs):
        # [16, 128]: row k=(hc*4+gg) holds bias[gg*512 + hc*128 : +128]
        blk = np.zeros((16, 128), np.float32)
        for hc in range(4):
            for g in range(4):
                blk[hc * 4 + g] = np.asarray(bs[g], np.float32)[
                    hc * 128:(hc + 1) * 128]
        return blk

    wf_np = aug([fWf, fWi, fWo, fWg])
    wb_np = aug([bWf, bWi, bWo, bWg])
    bias_np = np.stack([bias_block([fbf, fbi, fbo, fbg]),
                        bias_block([bbf, bbi, bbo, bbg])]).astype(bf)
    # indicator: column (bank, hcq, gg, t, b) belongs to row k=(2*bank+hcq)*4+gg
    ind_np = np.zeros((16, 1024), np.float32)
    for bank in range(2):
        for hcq in range(2):
            for g in range(4):
                k = (2 * bank + hcq) * 4 + g
                c0 = bank * 512 + hcq * 256 + g * 64
                ind_np[k, c0:c0 + 64] = 1.0
    ind_np = ind_np.astype(bf)
    aw_np = np.asarray(aW, np.float32).astype(bf)
    ab_np = np.asarray(ab, np.float32).reshape(8, 128).T.copy()
    av_np = np.asarray(av, np.float32).reshape(8, 128).T.astype(bf).copy()
    cw_np = np.asarray(cW, np.float32).copy()
    cb_np = np.tile(np.asarray(cb, np.float32), (BS, 1))

    x = np.asarray(x, np.float32)
    mask = np.asarray(mask)
    in_maps = []
    for i in range(NCORES):
        sl = slice(i * BS, (i + 1) * BS)
        xs0 = x[sl].transpose(1, 2, 0).astype(bf)          # [L, E, BS]
        # window-major: [NW, E, W, BS]
        xs = np.ascontiguousarray(
            xs0.reshape(NW, W, E, BS).transpose(0, 2, 1, 3))
        xsr = np.ascontiguousarray(
            xs0[::-1].reshape(NW, W, E, BS).transpose(0, 2, 1, 3))
        ma = ((mask[sl].astype(np.float32) - 1.0) * 1e9)
        in_maps.append({
            "xT": xs, "xTr": xsr, "wf": wf_np, "wb": wb_np,
            "bias_blk": bias_np, "ind": ind_np,
            "aw": aw_np, "ab_t": ab_np, "av_t": av_np,
            "cw": cw_np, "cb_rep": cb_np, "maskadd": ma,
        })
    return in_maps


def kernel(**inputs):
    from concourse.bass_utils import run_bass_kernel_spmd
    if "nc" not in _cache:
        _cache["nc"] = _build_nc()
    nc = _cache["nc"]
    in_maps = _prep_host(**inputs)
    res = run_bass_kernel_spmd(nc, in_maps, core_ids=list(range(NCORES)))
    return np.concatenate([res.results[i]["out"] for i in range(NCORES)],
                          axis=0)



# revision 3
# speedup vs baseline: 1.1498x; 1.1498x over previous
# Trainium2 Bass kernel for nn_EncoderRNN (bidirectional LSTM + attention +
# classifier).
#
# v2 sharding: direction-parallel x batch-parallel. Even cores run the forward
# LSTM, odd cores the backward LSTM (purely data-driven: the SPMD program is
# direction-agnostic; the host supplies reversed x / swapped weights). Each
# core scans BS=32 batch elements of ONE direction, which halves the
# tensor-engine LDWEIGHTS traffic per core vs computing both directions.
# The two directions of a batch block live on an SEngine-local core pair
# (2j, 2j+1); pairwise AllGathers (segmented, overlapped with the scan)
# exchange the halves needed for attention, which then runs data-parallel
# with 16 batch per core. A runtime register derived from partition_id
# selects the peer slot in the gathered buffer; time-reversal between the
# two directions is handled by staging the collective contribution
# time-flipped, so the whole program stays parity-symmetric.
#
# Recurrent weights are fp8e4m3 (host-quantized): LDWEIGHTS with FWL loads
# fp8 4 cols/cycle, halving the weight-load floor of the h-part matmuls.
# Numerics checked on host: rel err ~4.5e-4 vs the f64 oracle (tol 2e-2).
#
# Self-contained: hardcodes shapes; takes full inputs, returns full output.
import numpy as np
import ml_dtypes

B, L, E, H, C = 128, 512, 512, 512, 16
NCORES = 8
BS = 32                   # batch per core (one direction)
BA = 16                   # attention batch per core
W = 4                     # timesteps per x-precompute window
NW = L // W               # windows (128)
NSEG = 4                  # collective segments
QW = NW // NSEG           # windows per scan quarter (32)
SEGL = L // NSEG          # timesteps per segment (128)
UNROLL = 16               # For_i unroll factor
KC_H = H // 128           # h-part contraction chunks (4)
KC_E = E // 128           # x-part contraction chunks (4)
NMT = 4 * H // 128        # gate M-tiles (16); mt = gg*4 + hc
TOKCH = 16                # attention token chunks
TOKL = L // TOKCH         # l-range per token chunk (32)
PAIRS = [[0, 1], [2, 3], [4, 5], [6, 7]]
WH_FP8 = True

_cache = {}


def _build_nc():
    import concourse.bacc as bacc
    import concourse.mybir as mybir
    import concourse.tile as tile
    from concourse.bass import ds
    import contextlib

    f32 = mybir.dt.float32
    bf16 = mybir.dt.bfloat16
    whdt = mybir.dt.float8e4 if WH_FP8 else bf16
    AF = mybir.ActivationFunctionType
    ALU = mybir.AluOpType
    AX = mybir.AxisListType

    nc = bacc.Bacc("TRN2", target_bir_lowering=False, debug=False,
                   num_devices=NCORES)

    # ---- I/O ----
    xT = nc.dram_tensor("xT", [NW, E, W, BS], bf16, kind="ExternalInput").ap()
    wx = nc.dram_tensor("wx", [E, 4 * H], bf16, kind="ExternalInput").ap()
    wh = nc.dram_tensor("wh", [H, 4 * H], whdt, kind="ExternalInput").ap()
    bias_blk = nc.dram_tensor("bias_blk", [16, 128], bf16,
                              kind="ExternalInput").ap()
    ind = nc.dram_tensor("ind", [16, 2048], bf16, kind="ExternalInput").ap()
    aw = nc.dram_tensor("aw", [2 * H, 2 * H], bf16, kind="ExternalInput").ap()
    ab_t = nc.dram_tensor("ab_t", [128, 2 * H // 128], f32,
                          kind="ExternalInput").ap()
    av_t = nc.dram_tensor("av_t", [128, 2 * H // 128], bf16,
                          kind="ExternalInput").ap()
    cw = nc.dram_tensor("cw", [2 * H, C], f32, kind="ExternalInput").ap()
    cb_rep = nc.dram_tensor("cb_rep", [BA, C], f32, kind="ExternalInput").ap()
    maskadd = nc.dram_tensor("maskadd", [BA, L], f32,
                             kind="ExternalInput").ap()
    out = nc.dram_tensor("out", [BA, C], f32, kind="ExternalOutput").ap()

    # collective buffers: one pair per segment for exact dep tracking
    cc_in = [nc.dram_tensor(f"cc_in{s}", [4, 128, SEGL, BA], bf16).ap()
             for s in range(NSEG)]
    cc_out = [nc.dram_tensor(f"cc_out{s}", [2, 4, 128, SEGL, BA], bf16).ap()
              for s in range(NSEG)]

    with tile.TileContext(nc) as tc:
        with contextlib.ExitStack() as ctx:
            dramp = ctx.enter_context(tc.tile_pool(name="dram", bufs=1,
                                                   space="DRAM"))
            # local-half hidden states [hc][p, l, b] (b = first 16 of BS)
            hid = dramp.tile([4, 128, L, BA], bf16)
            alpha_d = dramp.tile([L, BA], f32)
            attw_d = dramp.tile([L, BA], bf16)

            # ================= Phase B: single-direction LSTM ==============
            with contextlib.ExitStack() as rctx:
                wpool = rctx.enter_context(tc.tile_pool(name="wp", bufs=1))
                xpool = rctx.enter_context(tc.tile_pool(name="xp", bufs=2))
                spool = rctx.enter_context(tc.tile_pool(name="sp", bufs=3))
                ppool = rctx.enter_context(
                    tc.tile_pool(name="pp", bufs=2, space="PSUM"))

                wx_sb = wpool.tile([128, KC_E, 4 * H], bf16, tag="wx")
                for kc in range(KC_E):
                    nc.sync.dma_start(out=wx_sb[:, kc, :],
                                      in_=wx[kc * 128:(kc + 1) * 128, :])
                wh_sb = wpool.tile([128, KC_H, 4 * H], whdt, tag="wh")
                for kc in range(KC_H):
                    nc.sync.dma_start(out=wh_sb[:, kc, :],
                                      in_=wh[kc * 128:(kc + 1) * 128, :])
                bb_sb = wpool.tile([16, 128], bf16, tag="bb")
                nc.sync.dma_start(out=bb_sb, in_=bias_blk)
                ind_sb = wpool.tile([16, 2048], bf16, tag="ind")
                nc.sync.dma_start(out=ind_sb, in_=ind)

                h_bf = wpool.tile([128, KC_H, BS], bf16, tag="h")
                nc.vector.memset(h_bf, 0.0)
                c_st = wpool.tile([128, KC_H, BS], f32, tag="c")
                nc.vector.memset(c_st, 0.0)

                stg_state = {}

                def window(wi, k, q):
                    if k % 2 == 0:
                        stg_state["l"] = spool.tile([128, KC_H, 2 * W, BA],
                                                    bf16, name="stgl",
                                                    tag="stgl")
                        stg_state["c"] = spool.tile([128, KC_H, 2 * W, BA],
                                                    bf16, name="stgc",
                                                    tag="stgc")
                    stgl, stgc = stg_state["l"], stg_state["c"]

                    x_sb = xpool.tile([128, KC_E, W, BS], bf16, tag="x")
                    for ec in range(KC_E):
                        nc.sync.dma_start(
                            out=x_sb[:, ec, :, :],
                            in_=xT[ds(wi, 1), ec * 128:(ec + 1) * 128,
                                   :, :].squeeze(0))
                    # psum [128, hc, gg, t, b] — bank b == h-chunk b
                    ps = ppool.tile([128, KC_H, 4, W, BS], f32, tag="ps")
                    psflat = ps.rearrange("p hc gg t b -> p (hc gg t b)")
                    for bank in range(4):
                        nc.tensor.matmul(
                            psflat[:, bank * 512:(bank + 1) * 512],
                            bb_sb[:, :],
                            ind_sb[:, bank * 512:(bank + 1) * 512],
                            start=True, stop=False, skip_group_check=True)
                    xflat = x_sb.rearrange("p e t b -> p e (t b)")
                    for ec in range(KC_E):
                        for mt in range(NMT):
                            gg, hc = mt // 4, mt % 4
                            nc.tensor.matmul(
                                ps[:, hc, gg, :, :],
                                wx_sb[:, ec, mt * 128:(mt + 1) * 128],
                                xflat[:, ec, :],
                                start=False, stop=False,
                                skip_group_check=True)

                    for ti in range(W):
                        for kc in range(KC_H):
                            for mt in range(NMT):
                                gg, hc = mt // 4, mt % 4
                                nc.tensor.matmul(
                                    ps[:, hc, gg, ti, :],
                                    wh_sb[:, kc, mt * 128:(mt + 1) * 128],
                                    h_bf[:, kc, :],
                                    start=False, stop=False,
                                    skip_group_check=True)
                        fio = spool.tile([128, KC_H, 3, BS], f32, tag="fio")
                        nc.scalar.activation(fio, ps[:, :, 0:3, ti, :],
                                             AF.Sigmoid)
                        g_s = spool.tile([128, KC_H, BS], f32, tag="g")
                        nc.scalar.activation(g_s, ps[:, :, 3, ti, :], AF.Tanh)
                        ig = spool.tile([128, KC_H, BS], f32, tag="ig")
                        nc.vector.tensor_mul(ig, fio[:, :, 1, :], g_s)
                        fc = spool.tile([128, KC_H, BS], f32, tag="fc")
                        nc.vector.tensor_mul(fc, fio[:, :, 0, :], c_st)
                        nc.vector.tensor_add(c_st, ig, fc)
                        tc_s = spool.tile([128, KC_H, BS], f32, tag="tc")
                        nc.scalar.activation(tc_s, c_st, AF.Tanh)
                        nc.vector.tensor_mul(h_bf, fio[:, :, 2, :], tc_s)
                        sl = (k % 2) * W + ti
                        nc.vector.tensor_copy(stgl[:, :, sl, :],
                                              h_bf[:, :, 0:BA])
                        # collective contribution staged time-flipped
                        nc.vector.tensor_copy(stgc[:, :, 2 * W - 1 - sl, :],
                                              h_bf[:, :, BA:BS])
                    if k % 2 == 1:
                        wbase = wi - 1
                        td0 = wbase * W
                        offc = (120 + 128 * q) - wbase * W
                        for hc in range(KC_H):
                            nc.sync.dma_start(
                                out=hid[hc, :, ds(td0, 2 * W), :],
                                in_=stgl[:, hc, :, :])
                            nc.sync.dma_start(
                                out=cc_in[3 - q][hc, :, ds(offc, 2 * W), :],
                                in_=stgc[:, hc, :, :])

                for q in range(NSEG):
                    def unroll_body(iv0, unroll, q=q):
                        assert unroll % 2 == 0, unroll
                        for k in range(unroll):
                            window(iv0 + k, k, q)

                    tc.For_i_unrolled_general(
                        q * QW, (q + 1) * QW, 1, unrollable_body=unroll_body,
                        max_unroll=UNROLL)
                    nc.gpsimd.collective_compute(
                        "AllGather", mybir.AluOpType.bypass,
                        replica_groups=PAIRS,
                        ins=[cc_in[3 - q].opt()],
                        outs=[cc_out[3 - q].opt()])

            # ================= Phase C: attention + classifier =============
            with contextlib.ExitStack() as actx:
                cpool = actx.enter_context(tc.tile_pool(name="cp", bufs=1))
                hpool = actx.enter_context(tc.tile_pool(name="hp", bufs=2))
                apool = actx.enter_context(tc.tile_pool(name="ap", bufs=3))
                mpool = actx.enter_context(tc.tile_pool(name="mp", bufs=1))
                pap = actx.enter_context(
                    tc.tile_pool(name="pap", bufs=2, space="PSUM"))
                pal = actx.enter_context(
                    tc.tile_pool(name="pal", bufs=2, space="PSUM"))

                peer = 1 - (nc.partition_id() & 1)

                aw_sb = cpool.tile([128, 8, 2 * H], bf16)
                for kc in range(8):
                    nc.sync.dma_start(out=aw_sb[:, kc, :],
                                      in_=aw[kc * 128:(kc + 1) * 128, :])
                ab_sb = cpool.tile([128, 8], f32)
                nc.sync.dma_start(out=ab_sb, in_=ab_t)
                av_sb = cpool.tile([128, 8], bf16)
                nc.sync.dma_start(out=av_sb, in_=av_t)

                # descending so the earliest-ready collective segment (3,
                # filled by scan quarter 0) is consumed first
                for tck in reversed(range(TOKCH)):
                    l0 = tck * TOKL
                    s = l0 // SEGL
                    lr = l0 - s * SEGL
                    hid_sb = hpool.tile([128, 8, TOKL, BA], bf16, tag="hsb")
                    for ch in range(4):
                        nc.sync.dma_start(out=hid_sb[:, ch, :, :],
                                          in_=hid[ch, :, l0:l0 + TOKL, :])
                    for ch in range(4):
                        nc.sync.dma_start(
                            out=hid_sb[:, 4 + ch, :, :],
                            in_=cc_out[s][ds(peer, 1), ch, :,
                                          lr:lr + TOKL, :].squeeze(0))
                    hflat = hid_sb.rearrange("p c l b -> p c (l b)")
                    ps_al = pal.tile([1, TOKL * BA], f32, tag="psal")
                    for m in range(8):
                        ps_a = pap.tile([128, TOKL * BA], f32, tag="psa")
                        for kc in range(8):
                            nc.tensor.matmul(
                                ps_a, aw_sb[:, kc, m * 128:(m + 1) * 128],
                                hflat[:, kc, :],
                                start=(kc == 0), stop=(kc == 7))
                        at_sb = apool.tile([128, TOKL * BA], bf16, tag="atsb")
                        nc.scalar.activation(at_sb, ps_a, AF.Tanh,
                                             bias=ab_sb[:, m:m + 1])
                        nc.tensor.matmul(ps_al, av_sb[:, m:m + 1], at_sb,
                                         start=(m == 0), stop=(m == 7))
                    al_sb = apool.tile([1, TOKL * BA], f32, tag="alsb")
                    nc.scalar.copy(al_sb, ps_al)
                    nc.sync.dma_start(
                        out=alpha_d[l0:l0 + TOKL, :],
                        in_=al_sb.rearrange("p (l b) -> p l b", l=TOKL))

                # softmax over l per b
                alv = mpool.tile([BA, L], f32)
                nc.sync.dma_start(out=alv, in_=alpha_d.rearrange("l b -> b l"))
                madd = mpool.tile([BA, L], f32)
                nc.sync.dma_start(out=madd, in_=maskadd)
                alm = mpool.tile([BA, L], f32)
                nc.vector.tensor_add(alm, alv, madd)
                negmax = mpool.tile([BA, 1], f32)
                nc.vector.tensor_reduce(negmax, alm, AX.X, ALU.max,
                                        negate=True)
                esb = mpool.tile([BA, L], f32)
                ssum = mpool.tile([BA, 1], f32)
                nc.scalar.activation(esb, alm, AF.Exp, bias=negmax,
                                     accum_out=ssum)
                rsum = mpool.tile([BA, 1], f32)
                nc.vector.reciprocal(rsum, ssum)
                attw = mpool.tile([BA, L], bf16)
                nc.vector.tensor_scalar_mul(attw, esb, rsum)
                nc.sync.dma_start(out=attw_d.rearrange("l b -> b l"), in_=attw)

                # sent = einsum over l
                import concourse.bass as bass
                attw_flat = attw_d.rearrange("l b -> (l b)")
                attw_bcast = bass.AP(tensor=attw_flat.tensor,
                                     offset=attw_flat.offset,
                                     ap=[[0, 128]] + list(attw_flat.ap))
                attw_rep = mpool.tile([128, L * BA], bf16)
                nc.sync.dma_start(out=attw_rep, in_=attw_bcast)
                arv = attw_rep.rearrange("p (l b) -> p l b", l=L)
                sent = mpool.tile([128, 8, BA], f32)
                for ch in range(8):
                    hfull = hpool.tile([128, L, BA], bf16, tag="hfull")
                    if ch < 4:
                        nc.sync.dma_start(out=hfull, in_=hid[ch, :, :, :])
                    else:
                        for s in range(NSEG):
                            nc.sync.dma_start(
                                out=hfull[:, s * SEGL:(s + 1) * SEGL, :],
                                in_=cc_out[s][ds(peer, 1), ch - 4, :,
                                              :, :].squeeze(0))
                    mul_t = hpool.tile([128, L, BA], bf16, tag="mult")
                    nc.vector.tensor_mul(mul_t, hfull, arv)
                    nc.vector.tensor_reduce(
                        sent[:, ch, :], mul_t.rearrange("p l b -> p b l"),
                        AX.X, ALU.add)

                # classifier
                cw_sb = cpool.tile([128, 8, C], f32)
                for kc in range(8):
                    nc.sync.dma_start(out=cw_sb[:, kc, :],
                                      in_=cw[kc * 128:(kc + 1) * 128, :])
                cb_sb = cpool.tile([BA, C], f32)
                nc.sync.dma_start(out=cb_sb, in_=cb_rep)
                sent_c = mpool.tile([128, 8, BA], f32)
                nc.vector.tensor_copy(sent_c, sent)
                ps_c = pal.tile([BA, C], f32, tag="psc")
                for ch in range(8):
                    nc.tensor.matmul(ps_c, sent_c[:, ch, :], cw_sb[:, ch, :],
                                     start=(ch == 0), stop=(ch == 7))
                logits = mpool.tile([BA, C], f32)
                nc.vector.tensor_add(logits, ps_c, cb_sb)
                ngm = mpool.tile([BA, 1], f32)
                nc.vector.tensor_reduce(ngm, logits, AX.X, ALU.max,
                                        negate=True)
                e2 = mpool.tile([BA, C], f32)
                s2 = mpool.tile([BA, 1], f32)
                nc.scalar.activation(e2, logits, AF.Exp, bias=ngm,
                                     accum_out=s2)
                lns = mpool.tile([BA, 1], f32)
                nc.scalar.activation(lns, s2, AF.Ln)
                tmp1 = mpool.tile([BA, C], f32)
                nc.vector.tensor_scalar_add(tmp1, logits, ngm)
                res = mpool.tile([BA, C], f32)
                nc.vector.tensor_scalar_sub(res, tmp1, lns)
                nc.sync.dma_start(out=out, in_=res)

    nc.compile()
    return nc


def _prep_host(x, mask, fWf, fbf, fWi, fbi, fWo, fbo, fWg, fbg,
               bWf, bbf, bWi, bbi, bWo, bbo, bWg, bbg,
               aW, ab, av, cW, cb):
    import concourse.mybir as mybir
    bf = ml_dtypes.bfloat16
    f8 = mybir.dt.np(mybir.dt.float8e4) if WH_FP8 else bf

    def wmat(Ws, r0, r1, dt):
        m = np.zeros((r1 - r0, 4 * H), np.float32)
        for g, Wg_ in enumerate(Ws):
            m[:, g * H:(g + 1) * H] = np.asarray(Wg_, np.float32)[r0:r1]
        return m.astype(dt)

    def bias_block(bs):
        blk = np.zeros((16, 128), np.float32)
        for hc in range(4):
            for g in range(4):
                blk[hc * 4 + g] = np.asarray(bs[g], np.float32)[
                    hc * 128:(hc + 1) * 128]
        return blk.astype(bf)

    fws = [fWf, fWi, fWo, fWg]
    bws = [bWf, bWi, bWo, bWg]
    wx_f = wmat(fws, 0, E, bf)
    wx_b = wmat(bws, 0, E, bf)
    wh_f = wmat(fws, E, E + H, f8)
    wh_b = wmat(bws, E, E + H, f8)
    bias_f = bias_block([fbf, fbi, fbo, fbg])
    bias_b = bias_block([bbf, bbi, bbo, bbg])

    ind_np = np.zeros((16, 2048), np.float32)
    for k in range(16):
        ind_np[k, k * 128:(k + 1) * 128] = 1.0
    ind_np = ind_np.astype(bf)

    aW_np = np.asarray(aW, np.float32)
    cW_np = np.asarray(cW, np.float32)
    aw_e = aW_np.astype(bf)
    aw_o = np.concatenate([aW_np[H:], aW_np[:H]], axis=0).astype(bf)
    cw_e = cW_np.copy()
    cw_o = np.concatenate([cW_np[H:], cW_np[:H]], axis=0)
    ab_np = np.asarray(ab, np.float32).reshape(8, 128).T.copy()
    av_np = np.asarray(av, np.float32).reshape(8, 128).T.astype(bf).copy()
    cb_np = np.tile(np.asarray(cb, np.float32), (BA, 1))

    x = np.asarray(x, np.float32)
    mask = np.asarray(mask)
    in_maps = []
    for c in range(NCORES):
        j, p = c // 2, c % 2
        if p == 0:
            bidx = np.arange(32 * j, 32 * j + 32)
        else:
            bidx = np.concatenate([np.arange(32 * j + 16, 32 * j + 32),
                                   np.arange(32 * j, 32 * j + 16)])
        xs0 = x[bidx].transpose(1, 2, 0).astype(bf)      # [L, E, BS]
        if p == 1:
            xs0 = xs0[::-1]
        xs = np.ascontiguousarray(
            xs0.reshape(NW, W, E, BS).transpose(0, 2, 1, 3))
        ma = ((mask[bidx[:BA]].astype(np.float32) - 1.0) * 1e9)
        if p == 1:
            ma = ma[:, ::-1].copy()
        in_maps.append({
            "xT": xs,
            "wx": wx_f if p == 0 else wx_b,
            "wh": wh_f if p == 0 else wh_b,
            "bias_blk": bias_f if p == 0 else bias_b,
            "ind": ind_np,
            "aw": aw_e if p == 0 else aw_o,
            "ab_t": ab_np, "av_t": av_np,
            "cw": cw_e if p == 0 else cw_o,
            "cb_rep": cb_np, "maskadd": ma,
        })
    return in_maps


def kernel(**inputs):
    from concourse.bass_utils import run_bass_kernel_spmd
    if "nc" not in _cache:
        _cache["nc"] = _build_nc()
    nc = _cache["nc"]
    in_maps = _prep_host(**inputs)
    res = run_bass_kernel_spmd(nc, in_maps, core_ids=list(range(NCORES)))
    full = np.zeros((B, C), np.float32)
    for c in range(NCORES):
        j, p = c // 2, c % 2
        b0 = 32 * j + 16 * p
        full[b0:b0 + BA] = res.results[c]["out"]
    return full
